# revision 32
# baseline (speedup 1.0000x reference)
"""Trainium2 Bass kernel for nn_Decoder (LSTM decoder + attention + lm_head).

Sharding: data-parallel over batch (64 -> 8 cores x 8). Each core runs the
full pipeline for its batch shard locally; no collectives.

Per-core pipeline (one NEFF):
  A) XGT = (X @ W_ih.T).T for all steps, feature-major bf16, step-major
     packed [128, t*128 + gcol] so each LSTM step reads one contiguous slice.
  B) 63 sequential LSTM cell steps, fully feature-major: per step one
     identity-matmul injects XGT[t] into PSUM, then 64 small matmuls
     (W_hh-chunk stationary [128,128] bf16 w/ FWL, h streaming N=8) accumulate
     h @ W_hh.T on top; f/i/o share one PSUM bank ([128,96]), g has its own
     (needs the x2 sigmoid scale). Pointwise on [128,32..96] tiles keeps all
     128 ACT/DVE lanes busy; h lands directly in the feature-major stores
     (bf16 via DVE - the next step's PE dependency - and f32 via ACT).
  C) Attention (f32): Q = W_in @ H.T; per batch element: scores via matmul
     with host-pretransposed encodings, masked exp via ACT bias, unnormalized
     ctx + denominator via matmuls, normalize with DVE reciprocal
  C2) Output projection (bf16) + tanh
  D) Vocab projection (bf16): logits = OUT @ W_lm.T, streamed over 32000
     vocab in 512-wide banks (single bank-packed DMA per bank, deep prefetch
     ring), fp16 PSUM eviction split across ACT/DVE, single packed store per
     bank. b_lm bias + fp32 upcast happen on the host during unshard.
"""
import sys

sys.path.insert(0, "/opt/trn_rl_repo")

import numpy as np
import ml_dtypes

from concourse import bacc, bass, mybir
from concourse.tile import TileContext
from concourse.bass_utils import run_bass_kernel_spmd

f32 = mybir.dt.float32
f16 = mybir.dt.float16
bf16 = mybir.dt.bfloat16
Act = mybir.ActivationFunctionType
Alu = mybir.AluOpType

NCORES = 8
T = 63            # decode steps (tgt_len - 1)
BL = 8            # batch per core
TOK = T * BL      # 504 tokens per core
TOKP = 512        # padded
SRC = 128
HID = 512
ENC = 512
INP = 512
V = 32000
GATES = 4 * HID   # 2048
NBANK = (V + 511) // 512  # 63 vocab banks (last = 256 wide, zero-padded)

# torch gate order i,f,g,o -> pipeline order f,i,o,g (f/i/o share a PSUM
# bank + one sigmoid; g is separate since tanh needs the x2 input scale)
PERM = np.concatenate([np.arange(512, 1024), np.arange(0, 512),
                       np.arange(1536, 2048), np.arange(1024, 1536)])

_BF = np.float16


def _build(niter: int = 1, phases: str = "ABCD", dbg: bool = False,
           small_out: bool = False) -> "bacc.Bacc":
    nc = bacc.Bacc("TRN2", target_bir_lowering=False)

    # xt packed [p, k*TOKP + tok] = X.T[k*128+p, tok]
    xt_d = nc.dram_tensor("xt", [128, 4 * TOKP], f16, kind="ExternalInput")
    # wih/whh packed [p, k*2048 + g] = W.T[k*128+p, g]
    wih_d = nc.dram_tensor("wih", [128, 4 * GATES], f16, kind="ExternalInput")
    whh_d = nc.dram_tensor("whh", [128, 4 * GATES], f16, kind="ExternalInput")
    h0t_d = nc.dram_tensor("h0t", [128, 32], f16, kind="ExternalInput")
    c0t_d = nc.dram_tensor("c0t", [128, 32], f32, kind="ExternalInput")
    id_d = nc.dram_tensor("id128", [128, 128], f16, kind="ExternalInput")
    # enc packed [s, b*512 + e] = enc[s, b, e]
    enc_d = nc.dram_tensor("encA", [128, BL * ENC], f16, kind="ExternalInput")
    # encT packed [p, b*512 + k*128 + s] = enc[s, b, k*128+p]
    enct_d = nc.dram_tensor("enctA", [128, BL * ENC], f16, kind="ExternalInput")
    mbt_d = nc.dram_tensor("mbt", [SRC, BL], f32, kind="ExternalInput")
    # wint packed [p, k*512 + e] = W_in.T[k*128+p, e]
    win_d = nc.dram_tensor("wint", [128, 4 * ENC], f16, kind="ExternalInput")
    # woutt packed [p, k*512 + e] = W_out.T[k*128+p, e], k in 0..7
    wout_d = nc.dram_tensor("woutt", [128, 8 * HID], f16, kind="ExternalInput")
    # W_lm.T bank-packed: [p, nb*2048 + k*512 + c] = W_lm.T[k*128+p, nb*512+c]
    wlm_d = nc.dram_tensor("wlm4", [128, NBANK * 2048], f16, kind="ExternalInput")
    if small_out:
        out_d = nc.dram_tensor("logits", [128, 2048], f16, kind="ExternalOutput")
    else:
        # bank-packed logits: [p, nb*2048 + mt*512 + c] = logits[mt*128+p, nb*512+c]
        out_d = nc.dram_tensor("logits", [128, NBANK * 2048], f16, kind="ExternalOutput")

    with TileContext(nc) as tc:
        for i in range(niter):
            if i:
                tc.strict_bb_all_engine_barrier()
            _emit_iter(nc, tc, xt_d, wih_d, whh_d, h0t_d, c0t_d, id_d, enc_d,
                       enct_d, mbt_d, win_d, wout_d, wlm_d, out_d,
                       phases=phases, dbg=dbg, small_out=small_out)
    nc.compile()
    return nc


def _emit_iter(nc, tc, xt_d, wih_d, whh_d, h0t_d, c0t_d, id_d, enc_d, enct_d,
               mbt_d, win_d, wout_d, wlm_d, out_d,
               phases: str = "ABCD", dbg: bool = False, small_out: bool = False):
    MM = nc.tensor.matmul

    def dump(dst_row, tiles, width=TOKP):
        # debug: copy tiles (any dtype) as f16 into logits[:, i*width..]
        with tc.tile_pool(name="dbg", bufs=2) as dp:
            for i, tl in enumerate(tiles):
                s = dp.tile([128, width], f16, tag="d", name="dbgt")
                nc.vector.tensor_copy(s[:, 0:width], tl[:, 0:width])
                nc.sync.dma_start(
                    out=out_d[:, i * width:(i + 1) * width], in_=s[:, 0:width])

    with (
        tc.tile_pool(name="const", bufs=1) as cp,
        tc.tile_pool(name="store", bufs=1) as stp,
        tc.tile_pool(name="pa_sb", bufs=1) as pa_sb,
    ):
        # ---- resident tiles (packed single-DMA loads), emitted in
        # consumption order: phase A first, then B, then C ----
        xtA = pa_sb.tile([128, 4 * TOKP], f16, tag="xtA", name="xtA")
        nc.sync.dma_start(out=xtA[:], in_=xt_d[:])
        xts = [xtA[:, k * TOKP:(k + 1) * TOKP] for k in range(4)]
        wihA = pa_sb.tile([128, 4 * GATES], f16, tag="wihA", name="wihA")
        nc.sync.dma_start(out=wihA[:], in_=wih_d[:])
        wih = [wihA[:, k * GATES:(k + 1) * GATES] for k in range(4)]
        whhA = cp.tile([128, 4 * GATES], f16, tag="whhA", name="whhA")
        nc.sync.dma_start(out=whhA[:], in_=whh_d[:])
        whh = [whhA[:, k * GATES:(k + 1) * GATES] for k in range(4)]
        h0t = cp.tile([128, 32], f16, tag="h0t", name="h0t")
        nc.sync.dma_start(out=h0t[:], in_=h0t_d[:])
        c0t = cp.tile([128, 32], f32, tag="c0t", name="c0t")
        nc.sync.dma_start(out=c0t[:], in_=c0t_d[:])
        id128 = cp.tile([128, 128], f16, tag="id128", name="id128")
        nc.sync.dma_start(out=id128[:], in_=id_d[:])
        mbt = cp.tile([SRC, BL], f32, tag="mbt", name="mbt")
        nc.sync.dma_start(out=mbt[:], in_=mbt_d[:])
        wintA = cp.tile([128, 4 * ENC], f16, tag="wintA", name="wintA")
        nc.sync.dma_start(out=wintA[:], in_=win_d[:])
        wint = [wintA[:, k * ENC:(k + 1) * ENC] for k in range(4)]
        wouttA = cp.tile([128, 8 * HID], f16, tag="wouttA", name="wouttA")
        nc.sync.dma_start(out=wouttA[:], in_=wout_d[:])
        woutt = [wouttA[:, k * HID:(k + 1) * HID] for k in range(8)]
        encA = cp.tile([128, BL * ENC], f16, tag="encA", name="encA")
        nc.sync.dma_start(out=encA[:], in_=enc_d[:])
        enc_sb = [encA[:, b * ENC:(b + 1) * ENC] for b in range(BL)]
        enctA = cp.tile([128, BL * ENC], f16, tag="enctA", name="enctA")
        nc.sync.dma_start(out=enctA[:], in_=enct_d[:])
        enct_sb = [[enctA[:, b * 512 + k * 128:b * 512 + (k + 1) * 128]
                    for k in range(4)] for b in range(BL)]
        ones_s = cp.tile([128, 1], f32, tag="ones_s", name="ones_s")
        nc.any.memset(ones_s[:], 1.0)
        ones_1 = cp.tile([1, 128], f32, tag="ones_1", name="ones_1")
        nc.any.memset(ones_1[:], 1.0)

        # ---- accumulating stores ----
        # XGT step-major: [p, t*128 + gcol], gcol = gc*8 + b (gc = G*4+m)
        xgtS = stp.tile([128, T * 128], f16, tag="xgtS", name="xgtS")
        htbB = stp.tile([128, 4 * TOKP], f16, tag="htbB", name="htbB")
        htb = [htbB[:, k * TOKP:(k + 1) * TOKP] for k in range(4)]
        # zero the TOK..TOKP padding so C2/D can run full-width tiles
        for k in range(4):
            nc.any.memset(htbB[:, k * TOKP + TOK:(k + 1) * TOKP], 0.0)
        qtf = [stp.tile([128, TOKP], f16, tag=f"qtf{m}", name=f"qtf{m}") for m in range(4)]
        htfbm = [stp.tile([128, TOKP], f16, tag=f"htfbm{k}", name=f"htfbm{k}") for k in range(4)]
        ctxt = [stp.tile([128, TOKP], f16, tag=f"ctxt{k}", name=f"ctxt{k}") for k in range(4)]
        outt = [stp.tile([128, TOKP], f16, tag=f"outt{m}", name=f"outt{m}") for m in range(4)]
        for k in range(4):
            nc.any.memset(ctxt[k][:, TOK:TOKP], 0.0)

        # ============ Phase A: XGT = (X @ W_ih.T).T feature-major ============
        if "A" not in phases:
            return
        xgtSv = xgtS[:].rearrange("p (t gcol) -> p t gcol", t=T)
        with (
            tc.tile_pool(name="pa_ps", bufs=2, space="PSUM") as pa_ps,
        ):
            for gc in range(16):
                ps = pa_ps.tile([128, TOK], f32, tag="pa", name="pa")
                for k in range(4):
                    MM(ps[:], wih[k][:, gc * 128:(gc + 1) * 128],
                       xts[k][:, 0:TOK], start=(k == 0), stop=(k == 3))
                # scatter token-major -> step-major bf16
                psv = ps[:].rearrange("p (t b) -> p t b", t=T)
                dst = xgtSv[:, :, gc * 8:(gc + 1) * 8]
                if gc % 2 == 0:
                    nc.vector.tensor_copy(dst, psv)
                else:
                    nc.scalar.copy(dst, psv)

        if dbg:
            dump(0, [xgtS[:, i * 2016:(i + 1) * 2016] for i in range(4)],
                 width=2016)
        # ================= Phase B: LSTM recurrence =================
        if "B" not in phases:
            return
        htbv = htbB[:].rearrange("p (k c) -> p k c", k=4)
        with (
            tc.tile_pool(name="pb_g", bufs=1, space="PSUM") as pb_g,
            tc.tile_pool(name="pb_tmp", bufs=2) as pb_tmp,
            tc.tile_pool(name="pb_c", bufs=2) as pb_c,
        ):
            c_prev = c0t
            # gate wave order f, i, g, o: the c-chain (needs f/i/g) completes
            # while the o-wave still runs on PE, so only sigo + the h-store
            # trail the matmul wave. PERM gate layout is f,i,o,g -> bank order
            # maps gate type F: psum bank BNK[F], xgtS column block XCOL[F].
            WAVE = (0, 1, 3, 2)   # f, i, g, o in PERM's f,i,o,g layout
            for t in range(T):
                # per-gate PSUM banks [128, 32]; col = m*8 + b
                ps = {F: pb_g.tile([128, 32], f32, tag=f"ps{F}", name=f"ps{F}")
                      for F in WAVE}
                x0 = t * 128
                for F in WAVE:
                    MM(ps[F][:], id128[:], xgtS[:, x0 + F * 32:x0 + (F + 1) * 32],
                       start=True, stop=False)
                for F in WAVE:
                    for m in range(4):
                        gc = F * 4 + m
                        o = ps[F][:, m * 8:(m + 1) * 8]
                        for k in range(4):
                            if t == 0:
                                rhs = h0t[:, k * 8:(k + 1) * 8]
                            else:
                                rhs = htbB[:, k * TOKP + (t - 1) * 8:
                                           k * TOKP + t * 8]
                            # stop only on the last MM into each PSUM bank —
                            # accumulation-group tracking is bank-granular
                            MM(o, whh[k][:, gc * 128:(gc + 1) * 128],
                               rhs, start=False, stop=(k == 3 and m == 3))
                # pointwise, all ACT ops are Sigmoid (tanh(x) = 2*sigmoid(2x)-1,
                # affine parts folded into DVE ops) to avoid ACT table swaps.
                sigf = pb_tmp.tile([128, 32], f32, tag="sigf", name="sigf")
                nc.scalar.activation(sigf[:], ps[0][:], Act.Sigmoid)
                c1 = pb_tmp.tile([128, 32], f32, tag="c1", name="c1")
                nc.vector.tensor_mul(c1[:], sigf[:], c_prev[:])
                sigi = pb_tmp.tile([128, 32], f32, tag="sigi", name="sigi")
                nc.scalar.activation(sigi[:], ps[1][:], Act.Sigmoid)
                tgs = pb_tmp.tile([128, 32], f32, tag="tgs", name="tgs")
                nc.scalar.activation(tgs[:], ps[3][:], Act.Sigmoid, scale=2.0)
                # up = (tgs - 0.5) * sigi  == sigi*tanh(g)/2
                up = pb_tmp.tile([128, 32], f32, tag="up", name="up")
                nc.vector.scalar_tensor_tensor(
                    up[:], tgs[:], 0.5, sigi[:],
                    op0=Alu.subtract, op1=Alu.mult)
                # c = c1 + 2*up
                c_new = pb_c.tile([128, 32], f32, tag="c", name="c")
                nc.vector.scalar_tensor_tensor(
                    c_new[:], up[:], 2.0, c1[:],
                    op0=Alu.mult, op1=Alu.add)
                tcs = pb_tmp.tile([128, 32], f32, tag="tcs", name="tcs")
                nc.scalar.activation(tcs[:], c_new[:], Act.Sigmoid, scale=2.0)
                sigo = pb_tmp.tile([128, 32], f32, tag="sigo", name="sigo")
                nc.scalar.activation(sigo[:], ps[2][:], Act.Sigmoid)
                # the store holds h/2 = (tcs - 0.5) * sigo; the missing x2 is
                # folded into W_hh / W_in / W_out (host-side pre-scaling).
                # On DVE: it's the next step's PE dependency.
                tcsv = tcs[:].rearrange("p (k b) -> p k b", k=4)
                sov = sigo[:].rearrange("p (k b) -> p k b", k=4)
                nc.vector.scalar_tensor_tensor(
                    htbv[:, :, t * 8:(t + 1) * 8], tcsv, 0.5, sov,
                    op0=Alu.subtract, op1=Alu.mult)
                c_prev = c_new

        if dbg:
            dump(128, htb, width=TOK)
        # ================= Phase C: attention (fp16 operands) =================
        if "C" not in phases:
            return
        with tc.tile_pool(name="pq_ps", bufs=2, space="PSUM") as pq_ps:
            # reorder H columns token-major -> b-major once, so every matmul
            # in the attention phase streams contiguous operands
            for k in range(4):
                hv = htb[k][:, 0:TOK].rearrange("p (j b) -> p b j", b=BL)
                bv = htfbm[k][:, 0:TOK].rearrange("p (b j) -> p b j", b=BL)
                nc.vector.tensor_copy(bv, hv)
            for m in range(4):
                ps = pq_ps.tile([128, TOK], f32, tag="q", name="q")
                for k in range(4):
                    MM(ps[:], wint[k][:, m * 128:(m + 1) * 128],
                       htfbm[k][:, 0:TOK], start=(k == 0), stop=(k == 3))
                nc.scalar.copy(qtf[m][:, 0:TOK], ps[:])

        TB = T * BL  # 504, b-major stage layout: col = b*T + j
        with (
            tc.tile_pool(name="pc_sb", bufs=1) as pc_sb,
            tc.tile_pool(name="pc_s", bufs=1, space="PSUM") as pc_s,
            tc.tile_pool(name="pc_d", bufs=1, space="PSUM") as pc_d,
            tc.tile_pool(name="pc_b", bufs=1, space="PSUM") as pc_b,
            tc.tile_pool(name="pc_c", bufs=1, space="PSUM") as pc_c,
        ):
            pss = pc_s.tile([SRC, TB], f32, tag="scores", name="scores")
            for b in range(BL):
                for k in range(4):
                    MM(pss[:, b * T:(b + 1) * T], enct_sb[b][k],
                       qtf[k][:, b * T:(b + 1) * T], start=(k == 0), stop=(k == 3))
            # exp stays f32 (values reach ~e^13); weights are normalized to
            # [0,1] BEFORE the ctx matmul so that matmul can run fp16
            e_all = pc_sb.tile([SRC, TB], f32, tag="e_all", name="e_all")
            for b in range(BL):
                nc.scalar.activation(e_all[:, b * T:(b + 1) * T],
                                     pss[:, b * T:(b + 1) * T], Act.Exp,
                                     bias=mbt[:, b:b + 1])
            psd = pc_d.tile([1, TB], f32, tag="denom", name="denom")
            MM(psd[:], ones_s[:], e_all[:], start=True, stop=True)
            rec = pc_sb.tile([1, TB], f32, tag="rec", name="rec")
            nc.vector.reciprocal(rec[:], psd[:])
            psb = pc_b.tile([128, TB], f32, tag="recb_ps", name="recb_ps")
            MM(psb[:], ones_1[:], rec[:], start=True, stop=True)
            attn = pc_sb.tile([SRC, TB], f16, tag="attn", name="attn")
            nc.vector.tensor_mul(attn[:], e_all[:], psb[:])
            for k in range(4):
                psc = pc_c.tile([128, TB], f32, tag=f"ctx{k}", name=f"ctx{k}")
                for b in range(BL):
                    MM(psc[:, b * T:(b + 1) * T],
                       enc_sb[b][:, k * 128:(k + 1) * 128],
                       attn[:, b * T:(b + 1) * T], start=True, stop=True)
                # scatter b-major -> token-major in one strided copy
                ctxv = ctxt[k][:, 0:TOK].rearrange("p (j b) -> p b j", b=BL)
                pscv = psc[:].rearrange("p (b j) -> p b j", b=BL)
                nc.vector.tensor_copy(ctxv, pscv)

        if dbg:
            dump(256, qtf, width=TOK)
            dump(384, ctxt, width=TOK)
        # ================= Phase C2: out-projection + tanh =================
        with tc.tile_pool(name="po_ps", bufs=2, space="PSUM") as po_ps:
            for m in range(4):
                ps = po_ps.tile([128, TOKP], f32, tag="o", name="o")
                for k in range(8):
                    rhs = ctxt[k] if k < 4 else htb[k - 4]
                    MM(ps[:], woutt[k][:, m * 128:(m + 1) * 128],
                       rhs[:, 0:TOKP], start=(k == 0), stop=(k == 7))
                nc.scalar.activation(outt[m][:, 0:TOKP], ps[:], Act.Tanh)

        if dbg:
            dump(0, outt, width=TOK)
        # ================= Phase D: vocab projection =================
        if "D" not in phases:
            return
        with (
            tc.tile_pool(name="pd_w", bufs=6) as pd_w,
            tc.tile_pool(name="pd_st", bufs=4) as pd_st,
            tc.tile_pool(name="pd_ps", bufs=4, space="PSUM") as pd_ps,
        ):
            for nb in range(NBANK):
                wl = pd_w.tile([128, 2048], f16, tag="wl", name="wl")
                nc.sync.dma_start(out=wl[:], in_=wlm_d[:, nb * 2048:(nb + 1) * 2048])
                st = pd_st.tile([128, 2048], f16, tag="st", name="st")
                for mt in range(4):
                    m0 = mt * 128
                    ps = pd_ps.tile([128, 512], f32, tag="v", name="v")
                    for k in range(4):
                        MM(ps[:], outt[k][:, m0:m0 + 128],
                           wl[:, k * 512:(k + 1) * 512],
                           start=(k == 0), stop=(k == 3))
                    # evictions split across ACT and DVE to balance engine load
                    dst = st[:, mt * 512:(mt + 1) * 512]
                    if mt % 2 == 0:
                        nc.scalar.copy(dst, ps[:])
                    else:
                        nc.vector.tensor_copy(dst, ps[:])
                dst = (out_d[:, 0:2048] if small_out
                       else out_d[:, nb * 2048:(nb + 1) * 2048])
                nc.sync.dma_start(out=dst, in_=st[:])


def _pack_k(a: np.ndarray, nk: int) -> np.ndarray:
    """[nk*128, N] -> [128, nk*N] with [p, k*N+c] = a[k*128+p, c]."""
    n = a.shape[1]
    return np.ascontiguousarray(
        a.reshape(nk, 128, n).transpose(1, 0, 2)).reshape(128, nk * n)


def _prep_in_maps(inputs: dict) -> list[dict]:
    targets = np.asarray(inputs["targets"])
    mask = np.asarray(inputs["attention_mask"])
    enc = np.asarray(inputs["encodings"], dtype=np.float32)
    h = np.asarray(inputs["h"], dtype=np.float32)
    c = np.asarray(inputs["c"], dtype=np.float32)
    emb = np.asarray(inputs["emb"], dtype=np.float32)
    W_ih = np.asarray(inputs["W_ih"], dtype=np.float32)
    W_hh = np.asarray(inputs["W_hh"], dtype=np.float32)
    W_in = np.asarray(inputs["W_in"], dtype=np.float32)
    W_out = np.asarray(inputs["W_out"], dtype=np.float32)
    W_lm = np.asarray(inputs["W_lm"], dtype=np.float32)

    x_seq = emb[targets[:-1]]                      # (63, 64, 512)
    # h is stored as h/2 on-device; compensate by doubling every weight
    # that consumes h (exact in fp: exponent bump only)
    wih_p = _pack_k(W_ih[PERM].T, 4).astype(_BF)   # (128, 4*2048)
    whh_p = _pack_k((2.0 * W_hh)[PERM].T, 4).astype(_BF)
    wint = _pack_k((2.0 * W_in).T, 4).astype(_BF)  # (128, 4*512)
    W_out2 = W_out.copy()
    W_out2[:, ENC:] *= 2.0                         # h-half of out_proj
    woutt = _pack_k(W_out2.T, 8).astype(_BF)       # (128, 8*512)
    # bank-packed W_lm.T: [p, nb*2048 + k*512 + c] = W_lm.T[k*128+p, nb*512+c]
    wp = np.zeros((512, NBANK * 512), np.float32)
    wp[:, :V] = W_lm.T
    wlm4 = np.ascontiguousarray(
        wp.reshape(4, 128, NBANK, 512).transpose(1, 2, 0, 3)
    ).reshape(128, NBANK * 2048).astype(_BF)
    id128 = np.eye(128, dtype=_BF)

    in_maps = []
    for cidx in range(NCORES):
        sl = slice(cidx * BL, (cidx + 1) * BL)
        xt = np.zeros((INP, TOKP), np.float32)
        xt[:, :TOK] = x_seq[:, sl, :].reshape(TOK, INP).T
        h0t = np.ascontiguousarray(h[sl].T / 2.0).reshape(4, 128, BL)  # h/2
        h0t = np.concatenate([h0t[k] for k in range(4)], axis=1)  # (128, 32)
        c0t = np.ascontiguousarray(c[sl].T).reshape(4, 128, BL)
        c0t = np.concatenate([c0t[k] for k in range(4)], axis=1)  # (128, 32)
        encc = enc[:, sl, :]                                      # (128, 8, 512)
        encA = np.ascontiguousarray(encc.transpose(0, 1, 2)).reshape(128, BL * ENC)
        enctA = np.ascontiguousarray(
            encc.reshape(SRC, BL, 4, 128).transpose(3, 1, 2, 0)
        ).reshape(128, BL * ENC)
        mbt = np.where(mask[:, sl], np.float32(-1e30), np.float32(0.0)).astype(np.float32)
        in_maps.append({
            "xt": _pack_k(xt, 4).astype(_BF),
            "wih": wih_p, "whh": whh_p,
            "h0t": h0t.astype(_BF),
            "c0t": c0t.astype(np.float32),
            "id128": id128,
            "encA": encA.astype(_BF),
            "enctA": enctA.astype(_BF),
            "mbt": mbt,
            "wint": wint, "woutt": woutt, "wlm4": wlm4,
        })
    return in_maps


def _assemble(results, b_lm: np.ndarray) -> np.ndarray:
    out = np.empty((T, 64, V), np.float32)
    bias = np.asarray(b_lm, dtype=np.float32)
    for cidx in range(NCORES):
        raw = np.asarray(results[cidx]["logits"])      # (128, NBANK*2048) f16
        lg = raw.reshape(128, NBANK, 4, 512).transpose(2, 0, 1, 3)
        lg = lg.reshape(512, NBANK * 512)[:TOK, :V].astype(np.float32)
        out[:, cidx * BL:(cidx + 1) * BL, :] = lg.reshape(T, BL, V) + bias
    return out


_CACHE: dict = {}


def kernel(**inputs) -> np.ndarray:
    if "nc" not in _CACHE:
        _CACHE["nc"] = _build(niter=1)
    in_maps = _prep_in_maps(inputs)
    res = run_bass_kernel_spmd(_CACHE["nc"], in_maps, core_ids=list(range(NCORES)))
    return _assemble(res.results, inputs["b_lm"])


# revision 34
# speedup vs baseline: 1.0703x; 1.0703x over previous
"""Trainium2 Bass kernel for nn_Decoder (LSTM decoder + attention + lm_head).

Sharding: data-parallel over batch (64 -> 8 cores x 8). Each core runs the
full pipeline for its batch shard locally; no collectives.

Per-core pipeline (one NEFF):
  A) XGT = (X @ W_ih.T).T for all steps, feature-major bf16, step-major
     packed [128, t*128 + gcol] so each LSTM step reads one contiguous slice.
  B) 63 sequential LSTM cell steps, fully feature-major: per step one
     identity-matmul injects XGT[t] into PSUM, then 64 small matmuls
     (W_hh-chunk stationary [128,128] bf16 w/ FWL, h streaming N=8) accumulate
     h @ W_hh.T on top; f/i/o share one PSUM bank ([128,96]), g has its own
     (needs the x2 sigmoid scale). Pointwise on [128,32..96] tiles keeps all
     128 ACT/DVE lanes busy; h lands directly in the feature-major stores
     (bf16 via DVE - the next step's PE dependency - and f32 via ACT).
  C) Attention (f32): Q = W_in @ H.T; per batch element: scores via matmul
     with host-pretransposed encodings, masked exp via ACT bias, unnormalized
     ctx + denominator via matmuls, normalize with DVE reciprocal
  C2) Output projection (bf16) + tanh
  D) Vocab projection (bf16): logits = OUT @ W_lm.T, streamed over 32000
     vocab in 512-wide banks (single bank-packed DMA per bank, deep prefetch
     ring), fp16 PSUM eviction split across ACT/DVE, single packed store per
     bank. b_lm bias + fp32 upcast happen on the host during unshard.
"""
import sys

sys.path.insert(0, "/opt/trn_rl_repo")

import numpy as np
import ml_dtypes

from concourse import bacc, bass, mybir
from concourse.tile import TileContext
from concourse.bass_utils import run_bass_kernel_spmd

f32 = mybir.dt.float32
f16 = mybir.dt.float16
bf16 = mybir.dt.bfloat16
Act = mybir.ActivationFunctionType
Alu = mybir.AluOpType

NCORES = 8
T = 63            # decode steps (tgt_len - 1)
BL = 8            # batch per core
TOK = T * BL      # 504 tokens per core
TOKP = 512        # padded
SRC = 128
HID = 512
ENC = 512
INP = 512
V = 32000
GATES = 4 * HID   # 2048
NBANK = (V + 511) // 512  # 63 vocab banks (last = 256 wide, zero-padded)

# torch gate order i,f,g,o -> pipeline order f,i,o,g (f/i/o share a PSUM
# bank + one sigmoid; g is separate since tanh needs the x2 input scale)
PERM = np.concatenate([np.arange(512, 1024), np.arange(0, 512),
                       np.arange(1536, 2048), np.arange(1024, 1536)])

_BF = np.float16


def _build(niter: int = 1, phases: str = "ABCD", dbg: bool = False,
           small_out: bool = False) -> "bacc.Bacc":
    nc = bacc.Bacc("TRN2", target_bir_lowering=False)

    # xt packed [p, k*TOKP + tok] = X.T[k*128+p, tok]
    xt_d = nc.dram_tensor("xt", [128, 4 * TOKP], f16, kind="ExternalInput")
    # wih/whh packed [p, k*2048 + g] = W.T[k*128+p, g]
    wih_d = nc.dram_tensor("wih", [128, 4 * GATES], f16, kind="ExternalInput")
    whh_d = nc.dram_tensor("whh", [128, 4 * GATES], f16, kind="ExternalInput")
    h0t_d = nc.dram_tensor("h0t", [128, 32], f16, kind="ExternalInput")
    c0t_d = nc.dram_tensor("c0t", [128, 32], f32, kind="ExternalInput")
    id_d = nc.dram_tensor("id128", [128, 128], f16, kind="ExternalInput")
    # enc packed [s, b*512 + e] = enc[s, b, e]
    enc_d = nc.dram_tensor("encA", [128, BL * ENC], f16, kind="ExternalInput")
    # encT packed [p, b*512 + k*128 + s] = enc[s, b, k*128+p]
    enct_d = nc.dram_tensor("enctA", [128, BL * ENC], f16, kind="ExternalInput")
    mbt_d = nc.dram_tensor("mbt", [SRC, BL], f32, kind="ExternalInput")
    # wint packed [p, k*512 + e] = W_in.T[k*128+p, e]
    win_d = nc.dram_tensor("wint", [128, 4 * ENC], f16, kind="ExternalInput")
    # woutt packed [p, k*512 + e] = W_out.T[k*128+p, e], k in 0..7
    wout_d = nc.dram_tensor("woutt", [128, 8 * HID], f16, kind="ExternalInput")
    # W_lm.T bank-packed: [p, nb*2048 + k*512 + c] = W_lm.T[k*128+p, nb*512+c]
    wlm_d = nc.dram_tensor("wlm4", [128, NBANK * 2048], f16, kind="ExternalInput")
    if small_out:
        out_d = nc.dram_tensor("logits", [128, 2048], f16, kind="ExternalOutput")
    else:
        # bank-packed logits: [p, nb*2048 + mt*512 + c] = logits[mt*128+p, nb*512+c]
        out_d = nc.dram_tensor("logits", [128, NBANK * 2048], f16, kind="ExternalOutput")

    with TileContext(nc) as tc:
        for i in range(niter):
            if i:
                tc.strict_bb_all_engine_barrier()
            _emit_iter(nc, tc, xt_d, wih_d, whh_d, h0t_d, c0t_d, id_d, enc_d,
                       enct_d, mbt_d, win_d, wout_d, wlm_d, out_d,
                       phases=phases, dbg=dbg, small_out=small_out)
    nc.compile()
    return nc


def _emit_iter(nc, tc, xt_d, wih_d, whh_d, h0t_d, c0t_d, id_d, enc_d, enct_d,
               mbt_d, win_d, wout_d, wlm_d, out_d,
               phases: str = "ABCD", dbg: bool = False, small_out: bool = False):
    MM = nc.tensor.matmul

    def dump(dst_row, tiles, width=TOKP):
        # debug: copy tiles (any dtype) as f16 into logits[:, i*width..]
        with tc.tile_pool(name="dbg", bufs=2) as dp:
            for i, tl in enumerate(tiles):
                s = dp.tile([128, width], f16, tag="d", name="dbgt")
                nc.vector.tensor_copy(s[:, 0:width], tl[:, 0:width])
                nc.sync.dma_start(
                    out=out_d[:, i * width:(i + 1) * width], in_=s[:, 0:width])

    with (
        tc.tile_pool(name="const", bufs=1) as cp,
        tc.tile_pool(name="store", bufs=1) as stp,
        tc.tile_pool(name="pa_sb", bufs=1) as pa_sb,
    ):
        # ---- resident tiles (packed single-DMA loads), emitted in
        # consumption order: phase A first, then B, then C ----
        xtA = pa_sb.tile([128, 4 * TOKP], f16, tag="xtA", name="xtA")
        nc.sync.dma_start(out=xtA[:], in_=xt_d[:])
        xts = [xtA[:, k * TOKP:(k + 1) * TOKP] for k in range(4)]
        wihA = pa_sb.tile([128, 4 * GATES], f16, tag="wihA", name="wihA")
        nc.sync.dma_start(out=wihA[:], in_=wih_d[:])
        wih = [wihA[:, k * GATES:(k + 1) * GATES] for k in range(4)]
        whhA = cp.tile([128, 4 * GATES], f16, tag="whhA", name="whhA")
        nc.sync.dma_start(out=whhA[:], in_=whh_d[:])
        whh = [whhA[:, k * GATES:(k + 1) * GATES] for k in range(4)]
        h0t = cp.tile([128, 32], f16, tag="h0t", name="h0t")
        nc.sync.dma_start(out=h0t[:], in_=h0t_d[:])
        c0t = cp.tile([128, 32], f32, tag="c0t", name="c0t")
        nc.sync.dma_start(out=c0t[:], in_=c0t_d[:])
        id128 = cp.tile([128, 128], f16, tag="id128", name="id128")
        nc.sync.dma_start(out=id128[:], in_=id_d[:])
        mbt = cp.tile([SRC, BL], f32, tag="mbt", name="mbt")
        nc.sync.dma_start(out=mbt[:], in_=mbt_d[:])
        wintA = cp.tile([128, 4 * ENC], f16, tag="wintA", name="wintA")
        nc.sync.dma_start(out=wintA[:], in_=win_d[:])
        wint = [wintA[:, k * ENC:(k + 1) * ENC] for k in range(4)]
        wouttA = cp.tile([128, 8 * HID], f16, tag="wouttA", name="wouttA")
        nc.sync.dma_start(out=wouttA[:], in_=wout_d[:])
        woutt = [wouttA[:, k * HID:(k + 1) * HID] for k in range(8)]
        encA = cp.tile([128, BL * ENC], f16, tag="encA", name="encA")
        nc.sync.dma_start(out=encA[:], in_=enc_d[:])
        enc_sb = [encA[:, b * ENC:(b + 1) * ENC] for b in range(BL)]
        enctA = cp.tile([128, BL * ENC], f16, tag="enctA", name="enctA")
        nc.sync.dma_start(out=enctA[:], in_=enct_d[:])
        enct_sb = [[enctA[:, b * 512 + k * 128:b * 512 + (k + 1) * 128]
                    for k in range(4)] for b in range(BL)]
        ones_s = cp.tile([128, 1], f32, tag="ones_s", name="ones_s")
        nc.any.memset(ones_s[:], 1.0)
        ones_1 = cp.tile([1, 128], f32, tag="ones_1", name="ones_1")
        nc.any.memset(ones_1[:], 1.0)

        # ---- accumulating stores ----
        # XGT step-major: [p, t*128 + gcol], gcol = gc*8 + b (gc = G*4+m)
        xgtS = stp.tile([128, T * 128], f16, tag="xgtS", name="xgtS")
        htbB = stp.tile([128, 4 * TOKP], f16, tag="htbB", name="htbB")
        htb = [htbB[:, k * TOKP:(k + 1) * TOKP] for k in range(4)]
        # zero the TOK..TOKP padding so C2/D can run full-width tiles
        for k in range(4):
            nc.any.memset(htbB[:, k * TOKP + TOK:(k + 1) * TOKP], 0.0)
        qtf = [stp.tile([128, TOKP], f16, tag=f"qtf{m}", name=f"qtf{m}") for m in range(4)]
        htfbm = [stp.tile([128, TOKP], f16, tag=f"htfbm{k}", name=f"htfbm{k}") for k in range(4)]
        ctxt = [stp.tile([128, TOKP], f16, tag=f"ctxt{k}", name=f"ctxt{k}") for k in range(4)]
        outt = [stp.tile([128, TOKP], f16, tag=f"outt{m}", name=f"outt{m}") for m in range(4)]
        for k in range(4):
            nc.any.memset(ctxt[k][:, TOK:TOKP], 0.0)

        # ============ Phase A: XGT = (X @ W_ih.T).T feature-major ============
        if "A" not in phases:
            return
        xgtSv = xgtS[:].rearrange("p (t gcol) -> p t gcol", t=T)
        with (
            tc.tile_pool(name="pa_ps", bufs=2, space="PSUM") as pa_ps,
        ):
            for gc in range(16):
                ps = pa_ps.tile([128, TOK], f32, tag="pa", name="pa")
                for k in range(4):
                    MM(ps[:], wih[k][:, gc * 128:(gc + 1) * 128],
                       xts[k][:, 0:TOK], start=(k == 0), stop=(k == 3))
                # scatter token-major -> step-major bf16
                psv = ps[:].rearrange("p (t b) -> p t b", t=T)
                dst = xgtSv[:, :, gc * 8:(gc + 1) * 8]
                if gc % 2 == 0:
                    nc.vector.tensor_copy(dst, psv)
                else:
                    nc.scalar.copy(dst, psv)

        if dbg:
            dump(0, [xgtS[:, i * 2016:(i + 1) * 2016] for i in range(4)],
                 width=2016)
        # ================= Phase B: LSTM recurrence =================
        if "B" not in phases:
            return
        htbv = htbB[:].rearrange("p (k c) -> p k c", k=4)
        with (
            tc.tile_pool(name="pb_g", bufs=2, space="PSUM") as pb_g,
            tc.tile_pool(name="pb_tmp", bufs=2) as pb_tmp,
            tc.tile_pool(name="pb_c", bufs=2) as pb_c,
        ):
            c_prev = c0t
            # gate wave order f, i, g, o: the c-chain (needs f/i/g) completes
            # while the o-wave still runs on PE, so only sigo + the h-store
            # trail the matmul wave. PERM gate layout is f,i,o,g -> bank order
            # maps gate type F: psum bank BNK[F], xgtS column block XCOL[F].
            WAVE = (0, 1, 3, 2)   # f, i, g, o in PERM's f,i,o,g layout
            for t in range(T):
                # per-gate PSUM banks [128, 32]; col = m*8 + b
                ps = {F: pb_g.tile([128, 32], f32, tag=f"ps{F}", name=f"ps{F}")
                      for F in WAVE}
                x0 = t * 128
                for F in WAVE:
                    MM(ps[F][:], id128[:], xgtS[:, x0 + F * 32:x0 + (F + 1) * 32],
                       start=True, stop=False)
                for F in WAVE:
                    for m in range(4):
                        gc = F * 4 + m
                        o = ps[F][:, m * 8:(m + 1) * 8]
                        for k in range(4):
                            if t == 0:
                                rhs = h0t[:, k * 8:(k + 1) * 8]
                            else:
                                rhs = htbB[:, k * TOKP + (t - 1) * 8:
                                           k * TOKP + t * 8]
                            # stop only on the last MM into each PSUM bank —
                            # accumulation-group tracking is bank-granular
                            MM(o, whh[k][:, gc * 128:(gc + 1) * 128],
                               rhs, start=False, stop=(k == 3 and m == 3))
                # pointwise, all ACT ops are Sigmoid (tanh(x) = 2*sigmoid(2x)-1,
                # affine parts folded into DVE ops) to avoid ACT table swaps.
                sigf = pb_tmp.tile([128, 32], f32, tag="sigf", name="sigf")
                nc.scalar.activation(sigf[:], ps[0][:], Act.Sigmoid)
                c1 = pb_tmp.tile([128, 32], f32, tag="c1", name="c1")
                nc.vector.tensor_mul(c1[:], sigf[:], c_prev[:])
                sigi = pb_tmp.tile([128, 32], f32, tag="sigi", name="sigi")
                nc.scalar.activation(sigi[:], ps[1][:], Act.Sigmoid)
                tgs = pb_tmp.tile([128, 32], f32, tag="tgs", name="tgs")
                nc.scalar.activation(tgs[:], ps[3][:], Act.Sigmoid, scale=2.0)
                # up = (tgs - 0.5) * sigi  == sigi*tanh(g)/2
                up = pb_tmp.tile([128, 32], f32, tag="up", name="up")
                nc.vector.scalar_tensor_tensor(
                    up[:], tgs[:], 0.5, sigi[:],
                    op0=Alu.subtract, op1=Alu.mult)
                # c = c1 + 2*up
                c_new = pb_c.tile([128, 32], f32, tag="c", name="c")
                nc.vector.scalar_tensor_tensor(
                    c_new[:], up[:], 2.0, c1[:],
                    op0=Alu.mult, op1=Alu.add)
                # sigo BEFORE tcs: ACT is strict FIFO, and tcs waits on the
                # DVE c-chain — sigo must not queue behind it
                sigo = pb_tmp.tile([128, 32], f32, tag="sigo", name="sigo")
                nc.scalar.activation(sigo[:], ps[2][:], Act.Sigmoid)
                tcs = pb_tmp.tile([128, 32], f32, tag="tcs", name="tcs")
                nc.scalar.activation(tcs[:], c_new[:], Act.Sigmoid, scale=2.0)
                # the store holds h/2 = (tcs - 0.5) * sigo; the missing x2 is
                # folded into W_hh / W_in / W_out (host-side pre-scaling).
                # On DVE: it's the next step's PE dependency.
                tcsv = tcs[:].rearrange("p (k b) -> p k b", k=4)
                sov = sigo[:].rearrange("p (k b) -> p k b", k=4)
                nc.vector.scalar_tensor_tensor(
                    htbv[:, :, t * 8:(t + 1) * 8], tcsv, 0.5, sov,
                    op0=Alu.subtract, op1=Alu.mult)
                c_prev = c_new

        if dbg:
            dump(128, htb, width=TOK)
        # ================= Phase C: attention (fp16 operands) =================
        if "C" not in phases:
            return
        with tc.tile_pool(name="pq_ps", bufs=2, space="PSUM") as pq_ps:
            # reorder H columns token-major -> b-major once, so every matmul
            # in the attention phase streams contiguous operands
            for k in range(4):
                hv = htb[k][:, 0:TOK].rearrange("p (j b) -> p b j", b=BL)
                bv = htfbm[k][:, 0:TOK].rearrange("p (b j) -> p b j", b=BL)
                nc.vector.tensor_copy(bv, hv)
            for m in range(4):
                ps = pq_ps.tile([128, TOK], f32, tag="q", name="q")
                for k in range(4):
                    MM(ps[:], wint[k][:, m * 128:(m + 1) * 128],
                       htfbm[k][:, 0:TOK], start=(k == 0), stop=(k == 3))
                nc.scalar.copy(qtf[m][:, 0:TOK], ps[:])

        TB = T * BL  # 504, b-major stage layout: col = b*T + j
        with (
            tc.tile_pool(name="pc_sb", bufs=1) as pc_sb,
            tc.tile_pool(name="pc_s", bufs=1, space="PSUM") as pc_s,
            tc.tile_pool(name="pc_d", bufs=1, space="PSUM") as pc_d,
            tc.tile_pool(name="pc_b", bufs=1, space="PSUM") as pc_b,
            tc.tile_pool(name="pc_c", bufs=1, space="PSUM") as pc_c,
        ):
            pss = pc_s.tile([SRC, TB], f32, tag="scores", name="scores")
            for b in range(BL):
                for k in range(4):
                    MM(pss[:, b * T:(b + 1) * T], enct_sb[b][k],
                       qtf[k][:, b * T:(b + 1) * T], start=(k == 0), stop=(k == 3))
            # exp stays f32 (values reach ~e^13); weights are normalized to
            # [0,1] BEFORE the ctx matmul so that matmul can run fp16
            e_all = pc_sb.tile([SRC, TB], f32, tag="e_all", name="e_all")
            for b in range(BL):
                nc.scalar.activation(e_all[:, b * T:(b + 1) * T],
                                     pss[:, b * T:(b + 1) * T], Act.Exp,
                                     bias=mbt[:, b:b + 1])
            psd = pc_d.tile([1, TB], f32, tag="denom", name="denom")
            MM(psd[:], ones_s[:], e_all[:], start=True, stop=True)
            rec = pc_sb.tile([1, TB], f32, tag="rec", name="rec")
            nc.vector.reciprocal(rec[:], psd[:])
            psb = pc_b.tile([128, TB], f32, tag="recb_ps", name="recb_ps")
            MM(psb[:], ones_1[:], rec[:], start=True, stop=True)
            attn = pc_sb.tile([SRC, TB], f16, tag="attn", name="attn")
            nc.vector.tensor_mul(attn[:], e_all[:], psb[:])
            for k in range(4):
                psc = pc_c.tile([128, TB], f32, tag=f"ctx{k}", name=f"ctx{k}")
                for b in range(BL):
                    MM(psc[:, b * T:(b + 1) * T],
                       enc_sb[b][:, k * 128:(k + 1) * 128],
                       attn[:, b * T:(b + 1) * T], start=True, stop=True)
                # scatter b-major -> token-major in one strided copy
                ctxv = ctxt[k][:, 0:TOK].rearrange("p (j b) -> p b j", b=BL)
                pscv = psc[:].rearrange("p (b j) -> p b j", b=BL)
                nc.vector.tensor_copy(ctxv, pscv)

        if dbg:
            dump(256, qtf, width=TOK)
            dump(384, ctxt, width=TOK)
        # ================= Phase C2: out-projection + tanh =================
        with tc.tile_pool(name="po_ps", bufs=2, space="PSUM") as po_ps:
            for m in range(4):
                ps = po_ps.tile([128, TOKP], f32, tag="o", name="o")
                for k in range(8):
                    rhs = ctxt[k] if k < 4 else htb[k - 4]
                    MM(ps[:], woutt[k][:, m * 128:(m + 1) * 128],
                       rhs[:, 0:TOKP], start=(k == 0), stop=(k == 7))
                nc.scalar.activation(outt[m][:, 0:TOKP], ps[:], Act.Tanh)

        if dbg:
            dump(0, outt, width=TOK)
        # ================= Phase D: vocab projection =================
        if "D" not in phases:
            return
        with (
            tc.tile_pool(name="pd_w", bufs=6) as pd_w,
            tc.tile_pool(name="pd_st", bufs=4) as pd_st,
            tc.tile_pool(name="pd_ps", bufs=4, space="PSUM") as pd_ps,
        ):
            for nb in range(NBANK):
                wl = pd_w.tile([128, 2048], f16, tag="wl", name="wl")
                nc.sync.dma_start(out=wl[:], in_=wlm_d[:, nb * 2048:(nb + 1) * 2048])
                st = pd_st.tile([128, 2048], f16, tag="st", name="st")
                for mt in range(4):
                    m0 = mt * 128
                    ps = pd_ps.tile([128, 512], f32, tag="v", name="v")
                    for k in range(4):
                        MM(ps[:], outt[k][:, m0:m0 + 128],
                           wl[:, k * 512:(k + 1) * 512],
                           start=(k == 0), stop=(k == 3))
                    # evictions split across ACT and DVE to balance engine load
                    dst = st[:, mt * 512:(mt + 1) * 512]
                    if mt % 2 == 0:
                        nc.scalar.copy(dst, ps[:])
                    else:
                        nc.vector.tensor_copy(dst, ps[:])
                dst = (out_d[:, 0:2048] if small_out
                       else out_d[:, nb * 2048:(nb + 1) * 2048])
                nc.sync.dma_start(out=dst, in_=st[:])


def _pack_k(a: np.ndarray, nk: int) -> np.ndarray:
    """[nk*128, N] -> [128, nk*N] with [p, k*N+c] = a[k*128+p, c]."""
    n = a.shape[1]
    return np.ascontiguousarray(
        a.reshape(nk, 128, n).transpose(1, 0, 2)).reshape(128, nk * n)


def _prep_in_maps(inputs: dict) -> list[dict]:
    targets = np.asarray(inputs["targets"])
    mask = np.asarray(inputs["attention_mask"])
    enc = np.asarray(inputs["encodings"], dtype=np.float32)
    h = np.asarray(inputs["h"], dtype=np.float32)
    c = np.asarray(inputs["c"], dtype=np.float32)
    emb = np.asarray(inputs["emb"], dtype=np.float32)
    W_ih = np.asarray(inputs["W_ih"], dtype=np.float32)
    W_hh = np.asarray(inputs["W_hh"], dtype=np.float32)
    W_in = np.asarray(inputs["W_in"], dtype=np.float32)
    W_out = np.asarray(inputs["W_out"], dtype=np.float32)
    W_lm = np.asarray(inputs["W_lm"], dtype=np.float32)

    x_seq = emb[targets[:-1]]                      # (63, 64, 512)
    # h is stored as h/2 on-device; compensate by doubling every weight
    # that consumes h (exact in fp: exponent bump only)
    wih_p = _pack_k(W_ih[PERM].T, 4).astype(_BF)   # (128, 4*2048)
    whh_p = _pack_k((2.0 * W_hh)[PERM].T, 4).astype(_BF)
    wint = _pack_k((2.0 * W_in).T, 4).astype(_BF)  # (128, 4*512)
    W_out2 = W_out.copy()
    W_out2[:, ENC:] *= 2.0                         # h-half of out_proj
    woutt = _pack_k(W_out2.T, 8).astype(_BF)       # (128, 8*512)
    # bank-packed W_lm.T: [p, nb*2048 + k*512 + c] = W_lm.T[k*128+p, nb*512+c]
    wp = np.zeros((512, NBANK * 512), np.float32)
    wp[:, :V] = W_lm.T
    wlm4 = np.ascontiguousarray(
        wp.reshape(4, 128, NBANK, 512).transpose(1, 2, 0, 3)
    ).reshape(128, NBANK * 2048).astype(_BF)
    id128 = np.eye(128, dtype=_BF)

    in_maps = []
    for cidx in range(NCORES):
        sl = slice(cidx * BL, (cidx + 1) * BL)
        xt = np.zeros((INP, TOKP), np.float32)
        xt[:, :TOK] = x_seq[:, sl, :].reshape(TOK, INP).T
        h0t = np.ascontiguousarray(h[sl].T / 2.0).reshape(4, 128, BL)  # h/2
        h0t = np.concatenate([h0t[k] for k in range(4)], axis=1)  # (128, 32)
        c0t = np.ascontiguousarray(c[sl].T).reshape(4, 128, BL)
        c0t = np.concatenate([c0t[k] for k in range(4)], axis=1)  # (128, 32)
        encc = enc[:, sl, :]                                      # (128, 8, 512)
        encA = np.ascontiguousarray(encc.transpose(0, 1, 2)).reshape(128, BL * ENC)
        enctA = np.ascontiguousarray(
            encc.reshape(SRC, BL, 4, 128).transpose(3, 1, 2, 0)
        ).reshape(128, BL * ENC)
        mbt = np.where(mask[:, sl], np.float32(-1e30), np.float32(0.0)).astype(np.float32)
        in_maps.append({
            "xt": _pack_k(xt, 4).astype(_BF),
            "wih": wih_p, "whh": whh_p,
            "h0t": h0t.astype(_BF),
            "c0t": c0t.astype(np.float32),
            "id128": id128,
            "encA": encA.astype(_BF),
            "enctA": enctA.astype(_BF),
            "mbt": mbt,
            "wint": wint, "woutt": woutt, "wlm4": wlm4,
        })
    return in_maps


def _assemble(results, b_lm: np.ndarray) -> np.ndarray:
    out = np.empty((T, 64, V), np.float32)
    bias = np.asarray(b_lm, dtype=np.float32)
    for cidx in range(NCORES):
        raw = np.asarray(results[cidx]["logits"])      # (128, NBANK*2048) f16
        lg = raw.reshape(128, NBANK, 4, 512).transpose(2, 0, 1, 3)
        lg = lg.reshape(512, NBANK * 512)[:TOK, :V].astype(np.float32)
        out[:, cidx * BL:(cidx + 1) * BL, :] = lg.reshape(T, BL, V) + bias
    return out


_CACHE: dict = {}


def kernel(**inputs) -> np.ndarray:
    if "nc" not in _CACHE:
        _CACHE["nc"] = _build(niter=1)
    in_maps = _prep_in_maps(inputs)
    res = run_bass_kernel_spmd(_CACHE["nc"], in_maps, core_ids=list(range(NCORES)))
    return _assemble(res.results, inputs["b_lm"])
